# revision 1
# baseline (speedup 1.0000x reference)
"""Trainium2 Bass kernel for multi-scale deformable attention (MSDeformAttn).

Self-contained: kernel(**inputs) takes the FULL unsharded inputs and returns
the FULL output. Internally shards data-parallel over batch (bs=16) across the
8 NeuronCores, runs a Bass/Tile kernel per core via run_bass_kernel_spmd, and
reassembles the output on host.
"""
import numpy as np

_CACHE = {}

# ======================================================================
# kernel builder (Bass/Tile)
# ======================================================================
import numpy as np
from contextlib import ExitStack

import concourse.bass as bass
import concourse.tile as tile
from concourse import mybir
from concourse.masks import make_identity

f32 = mybir.dt.float32
i32 = mybir.dt.int32
i16 = mybir.dt.int16
Alu = mybir.AluOpType
Act = mybir.ActivationFunctionType

VALUE_SHAPES = ((80, 80), (40, 40), (20, 20), (10, 10))
LV = 8500
NH, NL, NP, D, DH = 8, 4, 4, 256, 32
BASES = [0, 6400, 8000, 8400]
WLV = [80, 40, 20, 10]
SLOT = 64          # fp32 per position slot in v_perm (32 real + 32 pad)
NSLOT = LV + 2     # 8502


def host_const_tables():
    """Value-independent per-(h,l,p[,c]) constant rows, DMA-broadcast to 128
    partitions on device. Free layouts match the weight pipeline tiles."""
    # (h, l, p) width-128 tables
    wrep = np.zeros(128, np.float32)      # W_l
    krep = np.zeros(128, np.float32)      # base_l - 32*W_l - 31  (incl. +1 pad shift)
    for h in range(8):
        for l in range(4):
            for p in range(4):
                c = h * 16 + l * 4 + p
                wrep[c] = WLV[l]
                krep[c] = BASES[l] - 32 * WLV[l] - 31
    # (h, l, p, comp) width-256 tables for validity bounds (H_l == W_l here)
    whi0 = np.zeros(256, np.float32)      # W_l + 31  (corner0 upper, in x~ space)
    whi1 = np.zeros(256, np.float32)      # W_l + 30  (corner1 upper)
    wsc = np.zeros(256, np.float32)       # W_l (xy-wide scale)
    for h in range(8):
        for l in range(4):
            for p in range(4):
                for comp in range(2):
                    c = (h * 16 + l * 4 + p) * 2 + comp
                    whi0[c] = WLV[l] + 31
                    whi1[c] = WLV[l] + 30
                    wsc[c] = WLV[l]
    return {"wrep": wrep, "krep": krep, "whi0": whi0, "whi1": whi1, "wsc": wsc}


def build(nc, B=2, QP=1024, skip_gather=False):
    """Emit the kernel into nc (a Bacc). B batches/core, QP padded queries."""
    QC = QP // 128          # q-chunks
    ST = (LV + 127) // 128  # s-tiles for value projection (67)

    # ---------------- DRAM I/O ----------------
    queryT = nc.declare_dram_parameter("queryT", [B, D, QP], f32, isOutput=False)
    bbox = nc.declare_dram_parameter("bbox", [B, QP, 4], f32, isOutput=False)
    valueT = nc.declare_dram_parameter("valueT", [B, D, LV], f32, isOutput=False)
    W_cat = nc.declare_dram_parameter("W_cat", [D, 384], f32, isOutput=False)
    W_val = nc.declare_dram_parameter("W_val", [D, D], f32, isOutput=False)
    W_out = nc.declare_dram_parameter("W_out", [D, D], f32, isOutput=False)
    b_cat = nc.declare_dram_parameter("b_cat", [384], f32, isOutput=False)
    b_val = nc.declare_dram_parameter("b_val", [D], f32, isOutput=False)
    b_out = nc.declare_dram_parameter("b_out", [D], f32, isOutput=False)
    ctab = nc.declare_dram_parameter("ctab", [5, 256], f32, isOutput=False)
    outT = nc.declare_dram_parameter("outT", [B, D, QP], f32, isOutput=True)

    with tile.TileContext(nc) as tc, ExitStack() as ctx:
        # ---------------- pools ----------------
        const = ctx.enter_context(tc.tile_pool(name="const", bufs=1))
        dramp = ctx.enter_context(tc.tile_pool(name="dram", bufs=1, space="DRAM"))
        vload = ctx.enter_context(tc.tile_pool(name="vload", bufs=3))
        vout = ctx.enter_context(tc.tile_pool(name="vout", bufs=3))
        ab = ctx.enter_context(tc.tile_pool(name="ab", bufs=2))
        wtmp = ctx.enter_context(tc.tile_pool(name="wtmp", bufs=2))
        idxp = ctx.enter_context(tc.tile_pool(name="idxp", bufs=2))
        gat = ctx.enter_context(tc.tile_pool(name="gat", bufs=2))
        mres = ctx.enter_context(tc.tile_pool(name="mres", bufs=2))
        resp = ctx.enter_context(tc.tile_pool(name="resp", bufs=2))
        outp = ctx.enter_context(tc.tile_pool(name="outp", bufs=3))
        ps_v = ctx.enter_context(tc.tile_pool(name="ps_v", bufs=2, space="PSUM"))
        ps_a = ctx.enter_context(tc.tile_pool(name="ps_a", bufs=2, space="PSUM"))
        ps_w = ctx.enter_context(tc.tile_pool(name="ps_w", bufs=2, space="PSUM"))
        ps_o = ctx.enter_context(tc.tile_pool(name="ps_o", bufs=2, space="PSUM"))

        # ---------------- constants ----------------
        ident = const.tile([128, 128], f32)
        make_identity(nc, ident)
        # P_a[k, (g, r)] = 1 iff k == 16a + r: shift matrices whose output
        # replicates query rows 16a..16a+15 across all 8 16-partition groups
        pa_tiles = []
        for a in range(8):
            pa_t = const.tile([128, 8, 16], f32, tag=f"pa{a}")
            nc.gpsimd.memset(pa_t[:], 0.0)
            nc.gpsimd.affine_select(
                out=pa_t[:], in_=pa_t[:], compare_op=Alu.not_equal, fill=1.0,
                base=-16 * a, channel_multiplier=1, pattern=[[0, 8], [-1, 16]])
            pa_tiles.append(pa_t)
        zcol = const.tile([128, 1], f32)
        nc.vector.memset(zcol, 0.0)

        def brow(src_ap, n, name):
            """DMA-broadcast a DRAM row [n] to [128, n] sbuf tile."""
            t = const.tile([128, n], f32, tag=name)
            bc = bass.AP(tensor=src_ap.tensor, offset=src_ap.offset,
                         ap=[[0, 128]] + src_ap.ap)
            nc.sync.dma_start(out=t[:], in_=bc)
            return t

        bcat_rep = brow(b_cat[:], 384, "bcat")
        bv_rep = brow(b_val[:], 256, "bval")
        WREP = brow(ctab[0, :128], 128, "wrep")
        KREP = brow(ctab[1, :128], 128, "krep")
        WHI0 = brow(ctab[2, :], 256, "whi0")
        WHI1 = brow(ctab[3, :], 256, "whi1")
        WSC = brow(ctab[4, :], 256, "wsc")
        # b_out as per-partition scalars [128,1] x2
        bout_sb = const.tile([128, 2], f32)
        nc.sync.dma_start(
            out=bout_sb[:],
            in_=bass.AP(tensor=b_out[:].tensor, offset=0, ap=[[1, 128], [128, 2]]))
        # W_val / W_cat / W_out moving+stationary tiles (K halves)
        wval_sb = const.tile([128, 2, 256], f32)
        wcat_sb = const.tile([128, 2, 384], f32)
        wout_sb = const.tile([128, 2, 256], f32)
        for kh in range(2):
            nc.sync.dma_start(out=wval_sb[:, kh, :], in_=W_val[kh * 128:(kh + 1) * 128, :])
            nc.sync.dma_start(out=wcat_sb[:, kh, :], in_=W_cat[kh * 128:(kh + 1) * 128, :])
            nc.sync.dma_start(out=wout_sb[:, kh, :], in_=W_out[kh * 128:(kh + 1) * 128, :])

        # ---------------- v_perm DRAM scratch ----------------
        v_perm = dramp.tile([B, NH, NSLOT, SLOT], f32)
        # zero the two pad slots (gathered with zero weight; must not be NaN)
        zpad = const.tile([16, SLOT], f32)
        nc.vector.memset(zpad, 0.0)
        vp_ap = v_perm[:]
        for s in (0, NSLOT - 1):
            dst = vp_ap.rearrange("b h s d -> (b h) s d")[:, s, :]
            if B * NH <= 16:
                nc.sync.dma_start(out=dst, in_=zpad[:B * NH, :])
            else:
                raise AssertionError("B*NH > 16")

        # ---------------- value projection ----------------
        for b in range(B):
            for st in range(ST):
                s0 = st * 128
                sn = min(128, LV - s0)
                ps = ps_v.tile([128, 256], f32, tag="psv")
                for kh in range(2):
                    vt = vload.tile([128, 128], f32)
                    nc.sync.dma_start(
                        out=vt[:, :sn],
                        in_=valueT[b, kh * 128:(kh + 1) * 128, s0:s0 + sn])
                    nc.tensor.matmul(
                        ps[:sn, :], lhsT=vt[:, :sn], rhs=wval_sb[:, kh, :],
                        start=(kh == 0), stop=(kh == 1))
                vs = vout.tile([128, 256], f32)
                nc.scalar.activation(vs[:sn, :], ps[:sn, :], Act.Copy,
                                     bias=0.0, scale=1.0)
                # scatter to v_perm[b, :, 1+s0:1+s0+sn, :]; dh written twice
                # (slot pad half = copy of data half) so every byte is finite
                vsr = bass.AP(tensor=vs[:].tensor, offset=vs[:].offset,
                              ap=[[vs[:].ap[0][0], sn], [32, 8], [1, 32]])
                for r in range(2):
                    dst = vp_ap[b, :, 1 + s0:1 + s0 + sn,
                                r * 32:r * 32 + 32].rearrange("h s d -> s h d")
                    nc.sync.dma_start(out=dst, in_=vsr)

        # ---------------- per (b, qchunk) pipeline ----------------
        for b in range(B):
            for qc in range(QC):
                q0 = qc * 128
                # --- A: offsets+logits matmul ---
                pa = ps_a.tile([128, 384], f32, tag="psa")
                for kh in range(2):
                    qt = ab.tile([128, 128], f32, tag="qt")
                    nc.sync.dma_start(
                        out=qt[:],
                        in_=queryT[b, kh * 128:(kh + 1) * 128, q0:q0 + 128])
                    nc.tensor.matmul(pa[:], lhsT=qt[:], rhs=wcat_sb[:, kh, :],
                                     start=(kh == 0), stop=(kh == 1))
                AT = ab.tile([128, 384], f32, tag="AT")
                nc.vector.tensor_tensor(out=AT[:], in0=pa[:], in1=bcat_rep[:],
                                        op=Alu.add)
                # --- bbox scalars ---
                bb = ab.tile([128, 4], f32, tag="bb")
                nc.sync.dma_start(out=bb[:], in_=bbox[b, q0:q0 + 128, :])
                bbs = ab.tile([128, 2], f32, tag="bbs")   # w,h * 0.125
                nc.vector.tensor_scalar_mul(bbs[:], bb[:, 2:4], 0.125)
                # --- softmax over (l,p) per (q,h) ---
                E = ab.tile([128, 128], f32, tag="E")
                nc.scalar.activation(E[:], AT[:, 256:384], Act.Exp,
                                     bias=zcol[:], scale=1.0)
                Z = ab.tile([128, 8], f32, tag="Z")
                nc.vector.tensor_reduce(
                    out=Z[:], in_=E[:].rearrange("p (h g) -> p h g", g=16),
                    axis=mybir.AxisListType.X, op=Alu.add)
                R = ab.tile([128, 8], f32, tag="R")
                nc.vector.reciprocal(R[:], Z[:])
                AN = ab.tile([128, 128], f32, tag="AN")
                for h in range(8):
                    nc.vector.tensor_scalar_mul(
                        AN[:, h * 16:(h + 1) * 16], E[:, h * 16:(h + 1) * 16],
                        R[:, h:h + 1])
                # --- locations (xy-interleaved [128, 256] (h,l,p,c)) ---
                XY = AT[:, 0:256]
                U2 = wtmp.tile([128, 256], f32, tag="U2")
                u2v = U2[:].rearrange("p (k c) -> p k c", c=2)
                xyv = XY.rearrange("p (k c) -> p k c", c=2)
                for comp in range(2):
                    nc.vector.tensor_scalar(
                        out=u2v[:, :, comp], in0=xyv[:, :, comp],
                        scalar1=bbs[:, comp:comp + 1],
                        scalar2=bb[:, comp:comp + 1],
                        op0=Alu.mult, op1=Alu.add)
                XTR = wtmp.tile([128, 256], f32, tag="XTR")
                nc.vector.tensor_tensor(out=XTR[:], in0=U2[:], in1=WSC[:], op=Alu.mult)
                XT = wtmp.tile([128, 256], f32, tag="XT")   # x~ = loc*W + 31.5
                nc.vector.tensor_scalar_add(XT[:], XTR[:], 31.5)
                # floor via RNE round-trip + compare fix (mod unsupported)
                XN = wtmp.tile([128, 256], f32, tag="XN")
                nc.vector.tensor_scalar_add(XN[:], XT[:], 8388608.0)
                XN2 = wtmp.tile([128, 256], f32, tag="XN2")
                nc.vector.tensor_scalar_add(XN2[:], XN[:], -8388608.0)
                XG = wtmp.tile([128, 256], f32, tag="XG")
                nc.vector.tensor_tensor(out=XG[:], in0=XN2[:], in1=XT[:],
                                        op=Alu.is_gt)
                X0 = wtmp.tile([128, 256], f32, tag="X0")   # x0~ = floor(x~)
                nc.vector.tensor_tensor(out=X0[:], in0=XN2[:], in1=XG[:],
                                        op=Alu.subtract)
                FR = wtmp.tile([128, 256], f32, tag="FR")   # frac
                nc.vector.tensor_tensor(out=FR[:], in0=XT[:], in1=X0[:],
                                        op=Alu.subtract)
                # validity masks
                G0 = wtmp.tile([128, 256], f32, tag="G0")
                nc.vector.tensor_scalar(out=G0[:], in0=X0[:], scalar1=32.0,
                                        scalar2=None, op0=Alu.is_ge)
                H0 = wtmp.tile([128, 256], f32, tag="H0")
                nc.vector.tensor_tensor(out=H0[:], in0=X0[:], in1=WHI0[:],
                                        op=Alu.is_le)
                V0 = wtmp.tile([128, 256], f32, tag="V0")
                nc.vector.tensor_tensor(out=V0[:], in0=G0[:], in1=H0[:], op=Alu.mult)
                G1 = wtmp.tile([128, 256], f32, tag="G1")
                nc.vector.tensor_scalar(out=G1[:], in0=X0[:], scalar1=31.0,
                                        scalar2=None, op0=Alu.is_ge)
                H1 = wtmp.tile([128, 256], f32, tag="H1")
                nc.vector.tensor_tensor(out=H1[:], in0=X0[:], in1=WHI1[:],
                                        op=Alu.is_le)
                V1 = wtmp.tile([128, 256], f32, tag="V1")
                nc.vector.tensor_tensor(out=V1[:], in0=G1[:], in1=H1[:], op=Alu.mult)
                OMF = wtmp.tile([128, 256], f32, tag="OMF")  # 1 - frac
                nc.vector.tensor_scalar(out=OMF[:], in0=FR[:], scalar1=-1.0,
                                        scalar2=1.0, op0=Alu.mult, op1=Alu.add)
                # WV [128, 2(pos), 256(hlp,c)]: pos0=(1-f)*v0, pos1=f*v1
                WV = wtmp.tile([128, 2, 256], f32, tag="WV")
                nc.vector.tensor_tensor(out=WV[:, 0, :], in0=OMF[:], in1=V0[:],
                                        op=Alu.mult)
                nc.vector.tensor_tensor(out=WV[:, 1, :], in0=FR[:], in1=V1[:],
                                        op=Alu.mult)
                # A2 [128, 2(r), 128]: attn * wyv_r  (y comps are c=1 slices)
                A2 = wtmp.tile([128, 2, 128], f32, tag="A2")
                wvv = WV[:].rearrange("p r (k c) -> p r k c", c=2)
                for r in range(2):
                    nc.vector.tensor_tensor(out=A2[:, r, :], in0=AN[:],
                                            in1=wvv[:, r, :, 1], op=Alu.mult)
                # w4 [128, (h, lp, r, pos) = 512]
                W4 = wtmp.tile([128, 512], f32, tag="W4")
                w4v = W4[:].rearrange("p (h lp r pos) -> p h lp r pos",
                                      h=8, lp=16, r=2)
                in0 = bass.AP(tensor=A2[:].tensor, offset=A2[:].offset,
                              ap=[A2[:].ap[0], [16, 8], [1, 16], [128, 2], [0, 2]])
                in1 = bass.AP(tensor=WV[:].tensor, offset=WV[:].offset,
                              ap=[WV[:].ap[0], [32, 8], [2, 16], [0, 2], [256, 2]])
                nc.vector.tensor_tensor(out=w4v, in0=in0, in1=in1, op=Alu.mult)
                # --- indices ---
                # idx0 = y0~*W + x0~ + KREP ; idx1 = idx0 + WREP ; clamp [0, 8500]
                x0v = X0[:].rearrange("p (k c) -> p k c", c=2)
                IDX = idxp.tile([128, 2, 128], f32, tag="IDXF")
                nc.vector.tensor_tensor(out=IDX[:, 0, :], in0=x0v[:, :, 1],
                                        in1=WREP[:], op=Alu.mult)
                nc.vector.tensor_tensor(out=IDX[:, 0, :], in0=IDX[:, 0, :],
                                        in1=x0v[:, :, 0], op=Alu.add)
                nc.vector.tensor_tensor(out=IDX[:, 0, :], in0=IDX[:, 0, :],
                                        in1=KREP[:], op=Alu.add)
                nc.vector.tensor_tensor(out=IDX[:, 1, :], in0=IDX[:, 0, :],
                                        in1=WREP[:], op=Alu.add)
                IDXC = idxp.tile([128, 2, 128], f32, tag="IDXC")
                nc.vector.tensor_scalar(out=IDXC[:], in0=IDX[:], scalar1=0.0,
                                        scalar2=8500.0, op0=Alu.max, op1=Alu.min)
                # --- wrap to [16-part, (h, lp, r, a)] via 8 shift-matmuls ---
                WRF = idxp.tile([128, 8, 256], f32, tag="WRF")
                for a in range(8):
                    pw = ps_w.tile([128, 256], f32, tag="psw")
                    # rhs iterated (h, lp, r): strides in IDX free space:
                    # IDX free = [r(128), hlp(1)] -> dims h:16, lp:1, r:128
                    rhs = bass.AP(tensor=IDXC[:].tensor, offset=IDXC[:].offset,
                                  ap=[IDXC[:].ap[0], [16, 8], [1, 16], [128, 2]])
                    # lhsT = P_a: psum rows (g, r) <- q-row 16a+r (replicated
                    # on all 8 16-partition groups; the HW gather reads each
                    # gpsimd core's own partition group)
                    nc.tensor.matmul(pw[:], lhsT=pa_tiles[a][:].rearrange(
                        "p g r -> p (g r)"), rhs=rhs, start=True, stop=True)
                    # scatter into WRF[:, (h, lp, r), a]
                    dst = bass.AP(tensor=WRF[:].tensor,
                                  offset=WRF[:].offset + a,
                                  ap=[WRF[:].ap[0], [256, 8], [16, 16], [8, 2]])
                    nc.scalar.activation(dst, pw[:], Act.Copy, bias=0.0,
                                         scale=1.0)
                WRI = idxp.tile([128, 8, 256], i32, tag="WRI")
                nc.gpsimd.tensor_copy(out=WRI[:], in_=WRF[:])
                WRS = idxp.tile([128, 8, 256], i16, tag="WRS")
                nc.gpsimd.tensor_copy(out=WRS[:], in_=WRI[:])
                # --- per head: gather + weighted reduce ---
                RES = resp.tile([128, 256], f32, tag="RES")
                for h in range(8):
                    G = gat.tile([128, 32, 128], f32, tag="G")
                    if skip_gather:
                        nc.vector.memset(G[:], 1.0)
                    else:
                        src = bass.AP(tensor=vp_ap.tensor,
                                      offset=vp_ap.offset + (b * NH + h) * NSLOT * SLOT,
                                      ap=[[SLOT, NSLOT - 1], [1, 128]])
                        # SWDGE ring holds 1024 descriptors max per instruction
                        for k in range(4):
                            nc.gpsimd.dma_gather(
                                out_ap=G[:, 8 * k:8 * (k + 1), :], in_ap=src,
                                idxs_ap=WRS[:, h, 64 * k:64 * (k + 1)],
                                num_idxs=1024, num_idxs_reg=1024, elem_size=128,
                                elem_step=SLOT)
                    # m = g * w4[h]  (out dh-major [128, 32dh, 64j3])
                    M = mres.tile([128, 2048], f32, tag="M")
                    g_in = bass.AP(tensor=G[:].tensor, offset=G[:].offset,
                                   ap=[G[:].ap[0], [256, 16], [128, 2], [64, 2], [1, 32]])
                    w_in = bass.AP(tensor=W4[:].tensor, offset=W4[:].offset + h * 64,
                                   ap=[W4[:].ap[0], [4, 16], [2, 2], [1, 2], [0, 32]])
                    m_out = bass.AP(tensor=M[:].tensor, offset=M[:].offset,
                                    ap=[M[:].ap[0], [4, 16], [2, 2], [1, 2], [64, 32]])
                    nc.vector.tensor_tensor(out=m_out, in0=g_in, in1=w_in,
                                            op=Alu.mult)
                    # res[h] = reduce_j3(m)
                    nc.vector.tensor_reduce(
                        out=RES[:, h * 32:(h + 1) * 32],
                        in_=M[:].rearrange("p (d j) -> p d j", d=32),
                        axis=mybir.AxisListType.X, op=Alu.add)
                    # S = sum(w4[h]); res[h] += S * b_val[h]
                    Sh = mres.tile([128, 1], f32, tag="Sh")
                    nc.vector.tensor_reduce(
                        out=Sh[:], in_=W4[:, h * 64:(h + 1) * 64],
                        axis=mybir.AxisListType.X, op=Alu.add)
                    nc.vector.scalar_tensor_tensor(
                        out=RES[:, h * 32:(h + 1) * 32],
                        in0=bv_rep[:, h * 32:(h + 1) * 32], scalar=Sh[:],
                        in1=RES[:, h * 32:(h + 1) * 32],
                        op0=Alu.mult, op1=Alu.add)
                # --- out projection: outT[b,:,qc] = W_out^T @ RES^T + b_out ---
                rT = outp.tile([128, 2, 128], f32, tag="rT")
                for ch in range(2):
                    pt = ps_o.tile([128, 128], f32, tag="pso")
                    nc.tensor.transpose(pt[:], RES[:, ch * 128:(ch + 1) * 128],
                                        ident[:])
                    nc.scalar.activation(rT[:, ch, :], pt[:], Act.Copy,
                                         bias=0.0, scale=1.0)
                for co in range(2):
                    po = ps_o.tile([128, 128], f32, tag="pso")
                    for ch in range(2):
                        nc.tensor.matmul(
                            po[:], lhsT=wout_sb[:, ch, co * 128:(co + 1) * 128],
                            rhs=rT[:, ch, :], start=(ch == 0), stop=(ch == 1))
                    ob = outp.tile([128, 128], f32, tag="ob")
                    nc.scalar.activation(ob[:], po[:], Act.Identity,
                                         bias=bout_sb[:, co:co + 1], scale=1.0)
                    nc.sync.dma_start(out=outT[b, co * 128:(co + 1) * 128,
                                               q0:q0 + 128], in_=ob[:])
    return nc


def prep_in_map(inputs, batches, QP=1024):
    """Host-side prep: shard + transpose + pad for one core's batch slice."""
    q = inputs["query"][batches]            # [B, lq, 256]
    bb = inputs["refer_bbox"][batches][:, :, 0, :]  # [B, lq, 4]
    v = inputs["value"][batches]            # [B, lv, 256]
    B, lq, _ = q.shape
    n = min(lq, QP)
    qp = np.zeros((B, QP, D), np.float32)
    qp[:, :n] = q[:, :n]
    bbp = np.zeros((B, QP, 4), np.float32)
    bbp[:, :n] = bb[:, :n]
    ct = host_const_tables()
    ctab = np.zeros((5, 256), np.float32)
    ctab[0, :128] = ct["wrep"]
    ctab[1, :128] = ct["krep"]
    ctab[2] = ct["whi0"]
    ctab[3] = ct["whi1"]
    ctab[4] = ct["wsc"]
    return {
        "queryT": np.ascontiguousarray(qp.transpose(0, 2, 1)),
        "bbox": bbp,
        "valueT": np.ascontiguousarray(v.transpose(0, 2, 1)),
        "W_cat": np.ascontiguousarray(
            np.concatenate([inputs["W_off"], inputs["W_attn"]], axis=1)),
        "W_val": inputs["W_val"],
        "W_out": inputs["W_out"],
        "b_cat": np.ascontiguousarray(
            np.concatenate([inputs["b_off"], inputs["b_attn"]])),
        "b_val": inputs["b_val"],
        "b_out": inputs["b_out"],
        "ctab": ctab,
    }


def _get_compiled():
    if "nc" in _CACHE:
        return _CACHE["nc"]
    import concourse.bacc as bacc
    nc = bacc.Bacc()
    build(nc, B=2, QP=1024)
    nc.compile()
    _CACHE["nc"] = nc
    return nc


def kernel(**inputs):
    from concourse.bass_utils import run_bass_kernel_spmd
    nc = _get_compiled()
    in_maps = [prep_in_map(inputs, [2 * c, 2 * c + 1], QP=1024) for c in range(8)]
    res = run_bass_kernel_spmd(nc, in_maps, list(range(8)))
    bs, lq, d = inputs["query"].shape
    out = np.zeros((bs, lq, d), np.float32)
    for c in range(8):
        o = res.results[c]["outT"]          # [2, 256, 1024]
        out[2 * c:2 * c + 2] = o.transpose(0, 2, 1)[:, :lq, :]
    return out



# revision 18
# speedup vs baseline: 1.9039x; 1.9039x over previous
"""Trainium2 Bass kernel for multi-scale deformable attention (MSDeformAttn).

Self-contained: kernel(**inputs) takes the FULL unsharded inputs and returns
the FULL output. Internally shards data-parallel over batch (bs=16) across the
8 NeuronCores, runs a Bass/Tile kernel per core via run_bass_kernel_spmd, and
reassembles the output on host.

v2: paired-row value layout (v_pair[slot] = rows y,y+1 of one column) so a
single 512B gather descriptor fetches all 4 bilinear corners -- one
2048-descriptor gather per (batch, q-chunk, head), 4x fewer gather
instructions than the per-row baseline.
"""
import numpy as np

_CACHE = {}

# ======================================================================
# kernel builder (Bass/Tile)
# ======================================================================
from contextlib import ExitStack

import concourse.bass as bass
import concourse.tile as tile
from concourse import mybir
from concourse.masks import make_identity

f32 = mybir.dt.float32
i32 = mybir.dt.int32
i16 = mybir.dt.int16
Alu = mybir.AluOpType
Act = mybir.ActivationFunctionType

VALUE_SHAPES = ((80, 80), (40, 40), (20, 20), (10, 10))
LV = 8500
NH, NL, NP, D, DH = 8, 4, 4, 256, 32
BASES = [0, 6400, 8000, 8400]
WLV = [80, 40, 20, 10]
PAIR = 64          # fp32 per pair slot: rows (y, y+1) x 32 dh
# v_pair layout: slot 0 = zero pad, then per level: W pre-pad slots for the
# y0=-1 row (row1 half = level row 0, row0 half = zeros), then H*W data
# slots, then a trailing zero pad slot. DBASE[l] = slot of (y=0, x=0).
DBASE = [81, 6521, 8141, 8551]
NSLOT = 8652       # 1 + (150 pre-pad + 8500 data) + 1
CLAMP_MAX = 8650.0
NQ = 1             # SWDGE queues


def host_const_tables():
    """Value-independent per-(h,l,p[,c]) constant rows, DMA-broadcast to 128
    partitions on device. Free layouts match the weight pipeline tiles."""
    # (h, l, p) width-128 tables
    wrep = np.zeros(128, np.float32)      # W_l
    krep = np.zeros(128, np.float32)      # DBASE_l - 32*W_l - 32
    for h in range(8):
        for l in range(4):
            for p in range(4):
                c = h * 16 + l * 4 + p
                wrep[c] = WLV[l]
                krep[c] = DBASE[l] - 32 * WLV[l] - 32
    # (h, l, p, comp) width-256 tables for validity bounds (H_l == W_l here)
    whi0 = np.zeros(256, np.float32)      # W_l + 31  (corner0 upper, in x~ space)
    whi1 = np.zeros(256, np.float32)      # W_l + 30  (corner1 upper)
    wsc = np.zeros(256, np.float32)       # W_l (xy-wide scale)
    for h in range(8):
        for l in range(4):
            for p in range(4):
                for comp in range(2):
                    c = (h * 16 + l * 4 + p) * 2 + comp
                    whi0[c] = WLV[l] + 31
                    whi1[c] = WLV[l] + 30
                    wsc[c] = WLV[l]
    return {"wrep": wrep, "krep": krep, "whi0": whi0, "whi1": whi1, "wsc": wsc}


def build(nc, B=2, QP=1024, skip_gather=False):
    """Emit the kernel into nc (a Bacc). B batches/core, QP padded queries."""
    QC = QP // 128          # q-chunks

    # ---------------- DRAM I/O ----------------
    queryT = nc.declare_dram_parameter("queryT", [B, D, QP], f32, isOutput=False)
    bbox = nc.declare_dram_parameter("bbox", [B, QP, 4], f32, isOutput=False)
    valueT = nc.declare_dram_parameter("valueT", [B, D, LV], f32, isOutput=False)
    W_cat = nc.declare_dram_parameter("W_cat", [D, 384], f32, isOutput=False)
    W_val = nc.declare_dram_parameter("W_val", [D, D], f32, isOutput=False)
    W_out = nc.declare_dram_parameter("W_out", [D, D], f32, isOutput=False)
    b_cat = nc.declare_dram_parameter("b_cat", [384], f32, isOutput=False)
    b_val = nc.declare_dram_parameter("b_val", [D], f32, isOutput=False)
    b_out = nc.declare_dram_parameter("b_out", [D], f32, isOutput=False)
    ctab = nc.declare_dram_parameter("ctab", [5, 256], f32, isOutput=False)
    outT = nc.declare_dram_parameter("outT", [B, D, QP], f32, isOutput=True)

    with tile.TileContext(nc) as tc, ExitStack() as ctx:
        # ---------------- pools ----------------
        const = ctx.enter_context(tc.tile_pool(name="const", bufs=1))
        dramp = ctx.enter_context(tc.tile_pool(name="dram", bufs=1, space="DRAM"))
        vload = ctx.enter_context(tc.tile_pool(name="vload", bufs=3))
        vsp = ctx.enter_context(tc.tile_pool(name="vsp", bufs=3))
        ab = ctx.enter_context(tc.tile_pool(name="ab", bufs=2))
        wtmp = ctx.enter_context(tc.tile_pool(name="wtmp", bufs=2))
        idxp = ctx.enter_context(tc.tile_pool(name="idxp", bufs=2))
        gat = ctx.enter_context(tc.tile_pool(name="gat", bufs=3))
        mres = ctx.enter_context(tc.tile_pool(name="mres", bufs=2))
        resp = ctx.enter_context(tc.tile_pool(name="resp", bufs=2))
        outp = ctx.enter_context(tc.tile_pool(name="outp", bufs=3))
        ps_v = ctx.enter_context(tc.tile_pool(name="ps_v", bufs=2, space="PSUM"))
        ps_a = ctx.enter_context(tc.tile_pool(name="ps_a", bufs=2, space="PSUM"))
        ps_w = ctx.enter_context(tc.tile_pool(name="ps_w", bufs=2, space="PSUM"))
        ps_o = ctx.enter_context(tc.tile_pool(name="ps_o", bufs=2, space="PSUM"))

        # ---------------- constants ----------------
        ident = const.tile([128, 128], f32)
        make_identity(nc, ident)
        # P_a[k, (g, r)] = 1 iff k == 16a + r: shift matrices whose output
        # replicates query rows 16a..16a+15 across all 8 16-partition groups
        pa_tiles = []
        for a in range(8):
            pa_t = const.tile([128, 8, 16], f32, tag=f"pa{a}")
            nc.gpsimd.memset(pa_t[:], 0.0)
            nc.gpsimd.affine_select(
                out=pa_t[:], in_=pa_t[:], compare_op=Alu.not_equal, fill=1.0,
                base=-16 * a, channel_multiplier=1, pattern=[[0, 8], [-1, 16]])
            pa_tiles.append(pa_t)
        zcol = const.tile([128, 1], f32)
        nc.vector.memset(zcol, 0.0)
        zvs = const.tile([128, 256], f32)     # level-end flush source
        nc.vector.memset(zvs, 0.0)

        def brow(src_ap, n, name):
            """DMA-broadcast a DRAM row [n] to [128, n] sbuf tile."""
            t = const.tile([128, n], f32, tag=name)
            bc = bass.AP(tensor=src_ap.tensor, offset=src_ap.offset,
                         ap=[[0, 128]] + src_ap.ap)
            nc.sync.dma_start(out=t[:], in_=bc)
            return t

        bcat_rep = brow(b_cat[:], 384, "bcat")
        bv_rep = brow(b_val[:], 256, "bval")
        WREP = brow(ctab[0, :128], 128, "wrep")
        KREP = brow(ctab[1, :128], 128, "krep")
        WHI0 = brow(ctab[2, :], 256, "whi0")
        WHI1 = brow(ctab[3, :], 256, "whi1")
        WSC = brow(ctab[4, :], 256, "wsc")
        # b_out as per-partition scalars [128,1] x2
        bout_sb = const.tile([128, 2], f32)
        nc.sync.dma_start(
            out=bout_sb[:],
            in_=bass.AP(tensor=b_out[:].tensor, offset=0, ap=[[1, 128], [128, 2]]))
        # W_val / W_cat / W_out moving+stationary tiles (K halves)
        wval_sb = const.tile([128, 2, 256], f32)
        wcat_sb = const.tile([128, 2, 384], f32)
        wout_sb = const.tile([128, 2, 256], f32)
        for kh in range(2):
            nc.sync.dma_start(out=wval_sb[:, kh, :], in_=W_val[kh * 128:(kh + 1) * 128, :])
            nc.sync.dma_start(out=wcat_sb[:, kh, :], in_=W_cat[kh * 128:(kh + 1) * 128, :])
            nc.sync.dma_start(out=wout_sb[:, kh, :], in_=W_out[kh * 128:(kh + 1) * 128, :])

        # ---------------- v_pair DRAM scratch ----------------
        # v_pair[b, h, s, :] = [v[s], v[s + W_l]] (rows y, y+1 of column x)
        v_pair = dramp.tile([B, NH, NSLOT, PAIR], f32)
        # zero the two pad slots (gathered with zero weight; must not be NaN)
        zpad = const.tile([16, PAIR], f32)
        nc.vector.memset(zpad, 0.0)
        vp_ap = v_pair[:]
        for s in (0, NSLOT - 1):
            dst = vp_ap.rearrange("b h s d -> (b h) s d")[:, s, :]
            if B * NH <= 16:
                nc.sync.dma_start(out=dst, in_=zpad[:B * NH, :])
            else:
                raise AssertionError("B*NH > 16")

        # ---------------- value projection + pair build ----------------
        # v_pair[b,h,DBASE+s] = [v[s], v[s+W_l]]: each projected tile is
        # written twice -- row0 half at its own slots, row1 half at -W (which
        # also fills the level's y0=-1 pre-pad slots' row1 half).
        zvsv = zvs[:].rearrange("p (h d) -> p h d", h=8)
        for b in range(B):
            for l, (H, W) in enumerate(VALUE_SHAPES):
                sz = H * W
                db = DBASE[l]
                nt = (sz + 127) // 128
                for t in range(nt):
                    gs0 = BASES[l] + 128 * t
                    sn = min(128, sz - 128 * t)
                    ps = ps_v.tile([128, 256], f32, tag="psv")
                    for kh in range(2):
                        vt = vload.tile([128, 128], f32)
                        nc.sync.dma_start(
                            out=vt[:, :sn],
                            in_=valueT[b, kh * 128:(kh + 1) * 128, gs0:gs0 + sn])
                        nc.tensor.matmul(
                            ps[:sn, :], lhsT=vt[:, :sn], rhs=wval_sb[:, kh, :],
                            start=(kh == 0), stop=(kh == 1))
                    vs = vsp.tile([128, 256], f32, tag="vs")
                    nc.scalar.activation(vs[:sn, :], ps[:sn, :], Act.Copy,
                                         bias=0.0, scale=1.0)
                    vsv = vs[:].rearrange("p (h d) -> p h d", h=8)
                    s0 = db + 128 * t
                    # row0: v_pair[b, :, s0 : s0+sn, 0:32]
                    dst0 = vp_ap[b, :, s0:s0 + sn, 0:32].rearrange(
                        "h s d -> s h d")
                    nc.sync.dma_start(out=dst0, in_=vsv[:sn])
                    # row1: slot j holds v[j+W] -> write all rows at -W
                    dst1 = vp_ap[b, :, s0 - W:s0 - W + sn,
                                 32:64].rearrange("h s d -> s h d")
                    nc.sync.dma_start(out=dst1, in_=vsv[:sn])
                # pre-pad row0 (y0 == -1 corners, weight 0): zero-fill
                dstp = vp_ap[b, :, db - W:db, 0:32].rearrange("h s d -> s h d")
                nc.sync.dma_start(out=dstp, in_=zvsv[:W])
                # level tail: last W slots' row1 (y+1 == H) zero-filled
                dstz = vp_ap[b, :, db + sz - W:db + sz,
                             32:64].rearrange("h s d -> s h d")
                nc.sync.dma_start(out=dstz, in_=zvsv[:W])

        # ---------------- per (b, qchunk) pipeline ----------------
        for b in range(B):
            for qc in range(QC):
                q0 = qc * 128
                # --- A: offsets+logits matmul ---
                pa = ps_a.tile([128, 384], f32, tag="psa")
                for kh in range(2):
                    qt = ab.tile([128, 128], f32, tag="qt")
                    nc.sync.dma_start(
                        out=qt[:],
                        in_=queryT[b, kh * 128:(kh + 1) * 128, q0:q0 + 128])
                    nc.tensor.matmul(pa[:], lhsT=qt[:], rhs=wcat_sb[:, kh, :],
                                     start=(kh == 0), stop=(kh == 1))
                AT = ab.tile([128, 384], f32, tag="AT")
                nc.vector.tensor_tensor(out=AT[:], in0=pa[:], in1=bcat_rep[:],
                                        op=Alu.add)
                # --- bbox scalars ---
                bb = ab.tile([128, 4], f32, tag="bb")
                nc.sync.dma_start(out=bb[:], in_=bbox[b, q0:q0 + 128, :])
                bbs = ab.tile([128, 2], f32, tag="bbs")   # w,h * 0.125
                nc.vector.tensor_scalar_mul(bbs[:], bb[:, 2:4], 0.125)
                # --- softmax over (l,p) per (q,h) ---
                E = ab.tile([128, 128], f32, tag="E")
                nc.scalar.activation(E[:], AT[:, 256:384], Act.Exp,
                                     bias=zcol[:], scale=1.0)
                Z = ab.tile([128, 8], f32, tag="Z")
                nc.vector.tensor_reduce(
                    out=Z[:], in_=E[:].rearrange("p (h g) -> p h g", g=16),
                    axis=mybir.AxisListType.X, op=Alu.add)
                R = ab.tile([128, 8], f32, tag="R")
                nc.vector.reciprocal(R[:], Z[:])
                AN = ab.tile([128, 128], f32, tag="AN")
                for h in range(8):
                    nc.vector.tensor_scalar_mul(
                        AN[:, h * 16:(h + 1) * 16], E[:, h * 16:(h + 1) * 16],
                        R[:, h:h + 1])
                # --- locations (xy-interleaved [128, 256] (h,l,p,c)) ---
                XY = AT[:, 0:256]
                U2 = wtmp.tile([128, 256], f32, tag="U2")
                u2v = U2[:].rearrange("p (k c) -> p k c", c=2)
                xyv = XY.rearrange("p (k c) -> p k c", c=2)
                for comp in range(2):
                    nc.vector.tensor_scalar(
                        out=u2v[:, :, comp], in0=xyv[:, :, comp],
                        scalar1=bbs[:, comp:comp + 1],
                        scalar2=bb[:, comp:comp + 1],
                        op0=Alu.mult, op1=Alu.add)
                XTR = wtmp.tile([128, 256], f32, tag="XTR")
                nc.vector.tensor_tensor(out=XTR[:], in0=U2[:], in1=WSC[:], op=Alu.mult)
                XT = wtmp.tile([128, 256], f32, tag="XT")   # x~ = loc*W + 31.5
                nc.vector.tensor_scalar_add(XT[:], XTR[:], 31.5)
                # floor via RNE round-trip + compare fix (mod unsupported)
                XN = wtmp.tile([128, 256], f32, tag="XN")
                nc.vector.tensor_scalar_add(XN[:], XT[:], 8388608.0)
                XN2 = wtmp.tile([128, 256], f32, tag="XN2")
                nc.vector.tensor_scalar_add(XN2[:], XN[:], -8388608.0)
                XG = wtmp.tile([128, 256], f32, tag="XG")
                nc.vector.tensor_tensor(out=XG[:], in0=XN2[:], in1=XT[:],
                                        op=Alu.is_gt)
                X0 = wtmp.tile([128, 256], f32, tag="X0")   # x0~ = floor(x~)
                nc.vector.tensor_tensor(out=X0[:], in0=XN2[:], in1=XG[:],
                                        op=Alu.subtract)
                FR = wtmp.tile([128, 256], f32, tag="FR")   # frac
                nc.vector.tensor_tensor(out=FR[:], in0=XT[:], in1=X0[:],
                                        op=Alu.subtract)
                # validity masks
                G0 = wtmp.tile([128, 256], f32, tag="G0")
                nc.vector.tensor_scalar(out=G0[:], in0=X0[:], scalar1=32.0,
                                        scalar2=None, op0=Alu.is_ge)
                H0 = wtmp.tile([128, 256], f32, tag="H0")
                nc.vector.tensor_tensor(out=H0[:], in0=X0[:], in1=WHI0[:],
                                        op=Alu.is_le)
                V0 = wtmp.tile([128, 256], f32, tag="V0")
                nc.vector.tensor_tensor(out=V0[:], in0=G0[:], in1=H0[:], op=Alu.mult)
                G1 = wtmp.tile([128, 256], f32, tag="G1")
                nc.vector.tensor_scalar(out=G1[:], in0=X0[:], scalar1=31.0,
                                        scalar2=None, op0=Alu.is_ge)
                H1 = wtmp.tile([128, 256], f32, tag="H1")
                nc.vector.tensor_tensor(out=H1[:], in0=X0[:], in1=WHI1[:],
                                        op=Alu.is_le)
                V1 = wtmp.tile([128, 256], f32, tag="V1")
                nc.vector.tensor_tensor(out=V1[:], in0=G1[:], in1=H1[:], op=Alu.mult)
                OMF = wtmp.tile([128, 256], f32, tag="OMF")  # 1 - frac
                nc.vector.tensor_scalar(out=OMF[:], in0=FR[:], scalar1=-1.0,
                                        scalar2=1.0, op0=Alu.mult, op1=Alu.add)
                # WV [128, 2(pos), 256(hlp,c)]: pos0=(1-f)*v0, pos1=f*v1
                WV = wtmp.tile([128, 2, 256], f32, tag="WV")
                nc.vector.tensor_tensor(out=WV[:, 0, :], in0=OMF[:], in1=V0[:],
                                        op=Alu.mult)
                nc.vector.tensor_tensor(out=WV[:, 1, :], in0=FR[:], in1=V1[:],
                                        op=Alu.mult)
                # A2 [128, 2(dy), 128]: attn * wy_dy  (y comps are c=1 slices)
                A2 = wtmp.tile([128, 2, 128], f32, tag="A2")
                wvv = WV[:].rearrange("p r (k c) -> p r k c", c=2)
                for r in range(2):
                    nc.vector.tensor_tensor(out=A2[:, r, :], in0=AN[:],
                                            in1=wvv[:, r, :, 1], op=Alu.mult)
                # w4 [128, (h, lp, dx, dy) = 512]
                W4 = wtmp.tile([128, 512], f32, tag="W4")
                w4v = W4[:].rearrange("p (h lp dx dy) -> p h lp dx dy",
                                      h=8, lp=16, dx=2)
                in0 = bass.AP(tensor=A2[:].tensor, offset=A2[:].offset,
                              ap=[A2[:].ap[0], [16, 8], [1, 16], [0, 2], [128, 2]])
                in1 = bass.AP(tensor=WV[:].tensor, offset=WV[:].offset,
                              ap=[WV[:].ap[0], [32, 8], [2, 16], [256, 2], [0, 2]])
                nc.vector.tensor_tensor(out=w4v, in0=in0, in1=in1, op=Alu.mult)
                # --- indices: idx = y0~*W + x0~ + KREP; clamp valid ---
                x0v = X0[:].rearrange("p (k c) -> p k c", c=2)
                IDX = idxp.tile([128, 128], f32, tag="IDXF")
                nc.vector.tensor_tensor(out=IDX[:], in0=x0v[:, :, 1],
                                        in1=WREP[:], op=Alu.mult)
                nc.vector.tensor_tensor(out=IDX[:], in0=IDX[:],
                                        in1=x0v[:, :, 0], op=Alu.add)
                nc.vector.tensor_tensor(out=IDX[:], in0=IDX[:],
                                        in1=KREP[:], op=Alu.add)
                IDXC = idxp.tile([128, 128], f32, tag="IDXC")
                nc.vector.tensor_scalar(out=IDXC[:], in0=IDX[:], scalar1=0.0,
                                        scalar2=CLAMP_MAX, op0=Alu.max,
                                        op1=Alu.min)
                # --- wrap to [16-part, (h, lp, a)] via 8 shift-matmuls ---
                WRF = idxp.tile([128, 8, 128], f32, tag="WRF")
                for a in range(8):
                    pw = ps_w.tile([128, 128], f32, tag="psw")
                    # lhsT = P_a: psum rows (g, r) <- q-row 16a+r (replicated
                    # on all 8 16-partition groups; the HW gather reads each
                    # gpsimd core's own partition group)
                    nc.tensor.matmul(pw[:], lhsT=pa_tiles[a][:].rearrange(
                        "p g r -> p (g r)"), rhs=IDXC[:], start=True, stop=True)
                    # scatter into WRF[:, (h, lp), a]
                    dst = bass.AP(tensor=WRF[:].tensor,
                                  offset=WRF[:].offset + a,
                                  ap=[WRF[:].ap[0], [128, 8], [8, 16]])
                    nc.scalar.activation(dst, pw[:], Act.Copy, bias=0.0,
                                         scale=1.0)
                WRI = idxp.tile([128, 8, 128], i32, tag="WRI")
                nc.vector.tensor_copy(out=WRI[:], in_=WRF[:])
                WRS = idxp.tile([128, 8, 128], i16, tag="WRS")
                nc.vector.tensor_copy(out=WRS[:], in_=WRI[:])
                # --- per head: gather (prepare_only + trigger) + reduce ---
                RES = resp.tile([128, 256], f32, tag="RES")
                for h in range(8):
                    G = gat.tile([128, 16, 128], f32, tag="G")
                    if skip_gather:
                        nc.vector.memset(G[:], 1.0)
                    else:
                        src = bass.AP(
                            tensor=vp_ap.tensor,
                            offset=vp_ap.offset + (b * NH + h) * NSLOT * PAIR,
                            ap=[[PAIR, NSLOT - 1], [1, 128]])
                        # SWDGE ring holds 1024 descriptors per queue
                        # SWDGE ring: 1024 descriptors max per instruction
                        for k in range(2):
                            nc.gpsimd.dma_gather(
                                out_ap=G[:, 8 * k:8 * (k + 1), :], in_ap=src,
                                idxs_ap=WRS[:, h, 64 * k:64 * (k + 1)],
                                num_idxs=1024, num_idxs_reg=1024,
                                elem_size=128, elem_step=PAIR)
                    # m = g * w4[h]  (out dh-major [128, 32dh, 64j3])
                    M = mres.tile([128, 2048], f32, tag="M")
                    g_in = bass.AP(tensor=G[:].tensor, offset=G[:].offset,
                                   ap=[G[:].ap[0], [128, 16], [64, 2], [32, 2], [1, 32]])
                    w_in = bass.AP(tensor=W4[:].tensor, offset=W4[:].offset + h * 64,
                                   ap=[W4[:].ap[0], [4, 16], [2, 2], [1, 2], [0, 32]])
                    m_out = bass.AP(tensor=M[:].tensor, offset=M[:].offset,
                                    ap=[M[:].ap[0], [4, 16], [2, 2], [1, 2], [64, 32]])
                    nc.vector.tensor_tensor(out=m_out, in0=g_in, in1=w_in,
                                            op=Alu.mult)
                    # res[h] = reduce_j3(m)
                    nc.vector.tensor_reduce(
                        out=RES[:, h * 32:(h + 1) * 32],
                        in_=M[:].rearrange("p (d j) -> p d j", d=32),
                        axis=mybir.AxisListType.X, op=Alu.add)
                    # S = sum(w4[h]); res[h] += S * b_val[h]
                    Sh = mres.tile([128, 1], f32, tag="Sh")
                    nc.vector.tensor_reduce(
                        out=Sh[:], in_=W4[:, h * 64:(h + 1) * 64],
                        axis=mybir.AxisListType.X, op=Alu.add)
                    nc.vector.scalar_tensor_tensor(
                        out=RES[:, h * 32:(h + 1) * 32],
                        in0=bv_rep[:, h * 32:(h + 1) * 32], scalar=Sh[:],
                        in1=RES[:, h * 32:(h + 1) * 32],
                        op0=Alu.mult, op1=Alu.add)
                # --- out projection: outT[b,:,qc] = W_out^T @ RES^T + b_out ---
                rT = outp.tile([128, 2, 128], f32, tag="rT")
                for ch in range(2):
                    pt = ps_o.tile([128, 128], f32, tag="pso")
                    nc.tensor.transpose(pt[:], RES[:, ch * 128:(ch + 1) * 128],
                                        ident[:])
                    nc.scalar.activation(rT[:, ch, :], pt[:], Act.Copy,
                                         bias=0.0, scale=1.0)
                for co in range(2):
                    po = ps_o.tile([128, 128], f32, tag="pso")
                    for ch in range(2):
                        nc.tensor.matmul(
                            po[:], lhsT=wout_sb[:, ch, co * 128:(co + 1) * 128],
                            rhs=rT[:, ch, :], start=(ch == 0), stop=(ch == 1))
                    ob = outp.tile([128, 128], f32, tag="ob")
                    nc.scalar.activation(ob[:], po[:], Act.Identity,
                                         bias=bout_sb[:, co:co + 1], scale=1.0)
                    nc.sync.dma_start(out=outT[b, co * 128:(co + 1) * 128,
                                               q0:q0 + 128], in_=ob[:])
    return nc


def prep_in_map(inputs, batches, QP=1024):
    """Host-side prep: shard + transpose + pad for one core's batch slice."""
    q = inputs["query"][batches]            # [B, lq, 256]
    bb = inputs["refer_bbox"][batches][:, :, 0, :]  # [B, lq, 4]
    v = inputs["value"][batches]            # [B, lv, 256]
    B, lq, _ = q.shape
    n = min(lq, QP)
    qp = np.zeros((B, QP, D), np.float32)
    qp[:, :n] = q[:, :n]
    bbp = np.zeros((B, QP, 4), np.float32)
    bbp[:, :n] = bb[:, :n]
    ct = host_const_tables()
    ctab = np.zeros((5, 256), np.float32)
    ctab[0, :128] = ct["wrep"]
    ctab[1, :128] = ct["krep"]
    ctab[2] = ct["whi0"]
    ctab[3] = ct["whi1"]
    ctab[4] = ct["wsc"]
    return {
        "queryT": np.ascontiguousarray(qp.transpose(0, 2, 1)),
        "bbox": bbp,
        "valueT": np.ascontiguousarray(v.transpose(0, 2, 1)),
        "W_cat": np.ascontiguousarray(
            np.concatenate([inputs["W_off"], inputs["W_attn"]], axis=1)),
        "W_val": inputs["W_val"],
        "W_out": inputs["W_out"],
        "b_cat": np.ascontiguousarray(
            np.concatenate([inputs["b_off"], inputs["b_attn"]])),
        "b_val": inputs["b_val"],
        "b_out": inputs["b_out"],
        "ctab": ctab,
    }


def make_bacc():
    import concourse.bacc as bacc
    return bacc.Bacc(num_swdge_queues=NQ)


def _get_compiled():
    if "nc" in _CACHE:
        return _CACHE["nc"]
    nc = make_bacc()
    build(nc, B=2, QP=1024)
    nc.compile()
    _CACHE["nc"] = nc
    return nc


def kernel(**inputs):
    from concourse.bass_utils import run_bass_kernel_spmd
    nc = _get_compiled()
    in_maps = [prep_in_map(inputs, [2 * c, 2 * c + 1], QP=1024) for c in range(8)]
    res = run_bass_kernel_spmd(nc, in_maps, list(range(8)))
    bs, lq, d = inputs["query"].shape
    out = np.zeros((bs, lq, d), np.float32)
    for c in range(8):
        o = res.results[c]["outT"]          # [2, 256, 1024]
        out[2 * c:2 * c + 2] = o.transpose(0, 2, 1)[:, :lq, :]
    return out
